# revision 11
# baseline (speedup 1.0000x reference)
"""Multi-head attention (B=2, S=2048, D=1024, H=16) on 8 trn2 NeuronCores.

Sharding: core c -> batch b = c // 4, head group g = c % 4 (heads 4g..4g+3).
Each core computes, for its batch shard and 4 heads:
  QT/KT = (x W + b)^T in [d_local, seq] layout, V in [seq, d_local] layout,
  transposed scores S^T[k, q] = K Q^T (so softmax needs no transposes),
  exp via ACT (scale fused), fp8e4m3 DoubleRow PV with an appended ones
  column yielding both the unnormalized context and the softmax row sums,
  normalization via a gpsimd partition-broadcast reciprocal multiply, and
  a partial output projection against a row shard of Wo.
Host sums the 4 partials per batch and adds the constant row bv @ Wo + bo.

Schedule: the ACT engine (exp of 16.8M scores, ~1.11us per [128,1024]
unit) paces the kernel, and engines synchronize on cumulative completion
counters, so every PE instruction emitted between two scores units delays
the second unit's exp by its full duration. Hence:
  - per-head units are (token-half qh, key-chunk kc), scores first in
    each unit, fillers quantized to <=430ns pieces (half V projections,
    one projection chunk, 1-2 PV DoubleRow matmuls) so per-unit PE time
    stays at or under the ACT cadence;
  - PSUM: 4 banks of rotating [128,1024] scores/work tiles, 2 banks for
    the PV context accumulator (one token-half at a time - the exps of a
    whole head stay resident in fp8, bufs=16), and 2 banks for a
    dedicated projection-accumulator pool so the second-half/dblk1
    projections never wait on a normalize of the previous head;
  - PV for token-half 0 runs inside the head (pairs all ready by unit
    16); PV for half 1 + its normalize slide into the next head's first
    units; head 3's half-0 output blocks run inside head 3, so only 8
    output blocks + half-1 normalize remain after the last exp;
  - PSUM->SBUF output movers run on DVE in-head and alternate DVE/ACT in
    the tail; output DMA alternates the sync/gpsimd queues.
"""

import sys

sys.path.insert(0, "/opt/trn_rl_repo")

import numpy as np
import ml_dtypes

B = 2
S = 2048
D = 1024
H = 16
HD = 64
NCORES = 8
HPC = 4          # heads per core
DL = HPC * HD    # 256 local head dims per core
P = 128
KCH = S // P     # 16 key chunks
DCH = D // P     # 8 contraction chunks
TBLK = S // P    # 16 token blocks
NPAIR = KCH // 2
SCALE = 1.0 / np.sqrt(HD)

USE_FP8_PV = True

_CACHE = {}


def _build():
    import concourse.bass as bass  # noqa: F401
    import concourse.mybir as mybir
    import concourse.tile as tile
    from concourse import bacc

    bf16 = mybir.dt.bfloat16
    f32 = mybir.dt.float32
    fp8 = mybir.dt.float8e4
    DR = mybir.MatmulPerfMode.DoubleRow
    Exp = mybir.ActivationFunctionType.Exp

    nc = bacc.Bacc("TRN2", target_bir_lowering=False, debug=False,
                   num_devices=NCORES)

    xT_d = nc.dram_tensor("xt", [D, S], bf16, kind="ExternalInput")
    wq_d = nc.dram_tensor("wq", [D, DL], bf16, kind="ExternalInput")
    wk_d = nc.dram_tensor("wk", [D, DL], bf16, kind="ExternalInput")
    wv_d = nc.dram_tensor("wv", [D, DL], bf16, kind="ExternalInput")
    wo_d = nc.dram_tensor("wo", [DL, D], bf16, kind="ExternalInput")
    bqk_d = nc.dram_tensor("bqk", [P, 4], f32, kind="ExternalInput")
    out_d = nc.dram_tensor("out", [S, D], bf16, kind="ExternalOutput")

    att_dt = fp8 if USE_FP8_PV else bf16

    with tile.TileContext(nc) as tc:
        with (
            tc.tile_pool(name="persist", bufs=1) as pp,
            tc.tile_pool(name="stream", bufs=3) as sp,
            tc.tile_pool(name="psum", bufs=2, space="PSUM") as ps,
        ):
            # ---- input DMAs: weights first (small), then x token-halves.
            bqk_s = pp.tile([P, 4], f32, tag="bqk", name="bqk_s")
            nc.sync.dma_start(bqk_s[:], bqk_d[:])
            wq_s = pp.tile([P, DCH, DL], bf16, tag="wq", name="wq_s")
            wk_s = pp.tile([P, DCH, DL], bf16, tag="wk", name="wk_s")
            wv_s = pp.tile([P, DCH, DL], bf16, tag="wv", name="wv_s")
            xts = [pp.tile([P, S], bf16, tag=f"xt{c}", name=f"xt{c}")
                   for c in range(DCH)]
            # weight chunks interleaved with x first-halves so projection
            # matmuls can consume each chunk right as it lands
            nc.gpsimd.dma_start(wq_s[:, 0, :], wq_d[0:P, :])
            nc.sync.dma_start(wk_s[:, 0, :], wk_d[0:P, :])
            for c in range(DCH):
                eng = nc.sync if c % 2 == 0 else nc.gpsimd
                eng.dma_start(xts[c][:, 0:1024], xT_d[c * P:(c + 1) * P, 0:1024])
                if c + 1 < DCH:
                    nc.gpsimd.dma_start(wq_s[:, c + 1, :],
                                        wq_d[(c + 1) * P:(c + 2) * P, :])
                    nc.sync.dma_start(wk_s[:, c + 1, :],
                                      wk_d[(c + 1) * P:(c + 2) * P, :])
            for c in range(DCH):
                nc.gpsimd.dma_start(wv_s[:, c, :], wv_d[c * P:(c + 1) * P, :])
            for c in range(DCH):
                eng = nc.sync if c % 2 == 1 else nc.gpsimd
                eng.dma_start(xts[c][:, 1024:2048],
                              xT_d[c * P:(c + 1) * P, 1024:2048])
            wo_s = pp.tile([P, 2, D], bf16, tag="wo", name="wo_s")
            for dc in range(2):
                nc.gpsimd.dma_start(wo_s[:, dc, :], wo_d[dc * P:(dc + 1) * P, :])

            qt = [pp.tile([P, S], bf16, tag=f"qt{d}", name=f"qt{d}")
                  for d in range(2)]
            kt = [pp.tile([P, S], bf16, tag=f"kt{d}", name=f"kt{d}")
                  for d in range(2)]

            # ---- phase B: Q/K dblk0 first-token-half projections,
            # consuming each x half-chunk once on arrival.
            qacc = ps.tile([P, 1024], f32, tag="work", name="qacc_h0")
            kacc = ps.tile([P, 1024], f32, tag="work", name="kacc_h0")
            for c in range(DCH):
                for ns in range(2):
                    nc.tensor.matmul(
                        qacc[:, ns * 512:(ns + 1) * 512],
                        wq_s[:, c, 0:P],
                        xts[c][:, ns * 512:(ns + 1) * 512],
                        start=(c == 0), stop=(c == DCH - 1))
                for ns in range(2):
                    nc.tensor.matmul(
                        kacc[:, ns * 512:(ns + 1) * 512],
                        wk_s[:, c, 0:P],
                        xts[c][:, ns * 512:(ns + 1) * 512],
                        start=(c == 0), stop=(c == DCH - 1))
            nc.vector.tensor_scalar_add(qt[0][:, 0:1024], qacc[:],
                                        bqk_s[:, 0:1])
            nc.vector.tensor_scalar_add(kt[0][:, 0:1024], kacc[:],
                                        bqk_s[:, 2:3])

            # ---- projection-accumulator pool: its own 2 PSUM banks, one
            # proj-half at a time, 1 chunk (2 MMs) per unit.
            pj_ref = [None]

            def proj_step(which, dblk, half, c):
                w_s = wq_s if which == 0 else wk_s
                if c == 0:
                    pj_ref[0] = ps.tile([P, 1024], f32, tag="pacc", bufs=1,
                                        name=f"pacc{which}{dblk}{half}")
                acc = pj_ref[0]
                for ns in range(2):
                    nc.tensor.matmul(
                        acc[:, ns * 512:(ns + 1) * 512],
                        w_s[:, c, dblk * P:(dblk + 1) * P],
                        xts[c][:, half * 1024 + ns * 512:
                               half * 1024 + (ns + 1) * 512],
                        start=(c == 0), stop=(c == DCH - 1))
                if c == DCH - 1:
                    t_sb = qt[dblk] if which == 0 else kt[dblk]
                    bcol = dblk if which == 0 else 2 + dblk
                    nc.vector.tensor_scalar_add(
                        t_sb[:, half * 1024:(half + 1) * 1024],
                        acc[:], bqk_s[:, bcol:bcol + 1])

            # V tiles: k-chunk PAIRS [128, 2, 4 heads * 68]; col 68h+64 is
            # the softmax-sum ones column. Emitted as two 4-chunk quanta.
            vts = [None] * NPAIR
            v_ref = [None]

            def v_step(tb, quantum):
                pr, j = tb // 2, tb % 2
                if quantum == 0:
                    if j == 0:
                        vt = pp.tile([P, 2, HPC * 68], att_dt, tag=f"v{pr}",
                                     name=f"v{pr}")
                        v4 = vt.rearrange("p j (h e) -> p j h e", e=68)
                        nc.gpsimd.memset(v4[:, :, :, 64:65], 1.0)
                        vts[pr] = vt
                    v_ref[0] = ps.tile([P, 1024], f32, tag="work",
                                       name=f"ps_v{tb}")
                acc = v_ref[0]
                for kc in range(4 * quantum, 4 * quantum + 4):
                    nc.tensor.matmul(
                        acc[:, 0:DL],
                        xts[kc][:, tb * P:(tb + 1) * P],
                        wv_s[:, kc, :],
                        start=(kc == 0), stop=(kc == DCH - 1))
                if quantum == 1:
                    v4 = vts[pr].rearrange("p j (h e) -> p j h e", e=68)
                    nc.vector.tensor_copy(
                        v4[:, j, :, 0:64],
                        acc[:, 0:DL].rearrange("p (h e) -> p h e", e=64))

            etps = [None] * NPAIR
            ctx_ps = {}      # qh -> current [P,1024] ctx psum tile
            ctx_sb = [pp.tile([P, S], bf16, tag=f"ctx{dc}", name=f"ctx{dc}")
                      for dc in range(2)]

            def scores_unit(h, kc, qh):
                dblk = h // 2
                roff = 64 * (h % 2)
                pr, j = kc // 2, kc % 2
                if j == 0 and qh == 0:
                    # whole head resident: 16 buffers so PV of half 1 can
                    # trail into the next head without stalling allocs.
                    etps[pr] = sp.tile([P, 2, S], att_dt, tag="expt",
                                       bufs=16, name=f"expt{h}_{pr}")
                sc = ps.tile([P, 1024], f32, tag="work",
                             name=f"sc{h}_{kc}_{qh}")
                for ns in range(2):
                    nc.tensor.matmul(
                        sc[:, ns * 512:(ns + 1) * 512],
                        kt[dblk][roff:roff + 64, kc * P:(kc + 1) * P],
                        qt[dblk][roff:roff + 64,
                                 qh * 1024 + ns * 512:qh * 1024 + (ns + 1) * 512],
                        start=True, stop=True)
                nc.scalar.activation(
                    etps[pr][:, j, qh * 1024:(qh + 1) * 1024], sc[:],
                    Exp, scale=float(SCALE))

            def pv_step(h, qh, pr, ets=None):
                """One pair's PV for one token-half: 2 fp8 DR matmuls."""
                if pr == 0:
                    ctx_ps[qh] = ps.tile([P, 1024], f32, tag="ctx", bufs=1,
                                         name=f"ps_ctx{h}_{qh}")
                cps = ctx_ps[qh]
                v4 = vts[pr].rearrange("p j (h e) -> p j h e", e=68)
                et = ets if ets is not None else etps[pr]
                for ns in range(2):
                    nc.tensor.matmul(
                        cps[0:65, ns * 512:(ns + 1) * 512],
                        v4[:, :, h, 0:65],
                        et[:, :, qh * 1024 + ns * 512:
                           qh * 1024 + (ns + 1) * 512],
                        start=(pr == 0), stop=(pr == NPAIR - 1),
                        perf_mode=DR)

            def normalize(h, qh, part, nparts=2):
                """Normalize 1/nparts of head h's half-qh context."""
                dblk = h // 2
                roff = 64 * (h % 2)
                w = 1024 // nparts
                cps = ctx_ps[qh]
                hs = slice(part * w, (part + 1) * w)
                gs = slice(qh * 1024 + part * w, qh * 1024 + (part + 1) * w)
                srow = sp.tile([1, w], f32, tag=f"srow{w}", bufs=2,
                               name=f"srow{h}_{qh}_{part}")
                nc.vector.tensor_copy(srow[:], cps[64:65, hs])
                rec = sp.tile([1, w], f32, tag=f"rec{w}", bufs=2,
                              name=f"rec{h}_{qh}_{part}")
                nc.vector.reciprocal_approx_fast(rec[:], srow[:])
                bc = sp.tile([64, w], f32, tag=f"bc{w}", bufs=2,
                             name=f"bc{h}_{qh}_{part}")
                nc.gpsimd.partition_broadcast(bc[:], rec[:])
                nc.vector.tensor_mul(
                    ctx_sb[dblk][roff:roff + 64, gs],
                    cps[0:64, hs], bc[:])

            def out_tb(tb, mover):
                acc = ps.tile([P, 1024], f32, tag="work", name=f"ps_o{tb}")
                for dc in range(2):
                    for ns in range(2):
                        nc.tensor.matmul(
                            acc[:, ns * 512:(ns + 1) * 512],
                            ctx_sb[dc][:, tb * P:(tb + 1) * P],
                            wo_s[:, dc, ns * 512:(ns + 1) * 512],
                            start=(dc == 0), stop=(dc == 1))
                o_sb = sp.tile([P, D], bf16, tag="osb", name=f"osb{tb}")
                if mover == 0:
                    nc.vector.tensor_copy(o_sb[:], acc[:])
                else:
                    nc.scalar.copy(o_sb[:], acc[:])
                eng = nc.sync if tb % 2 == 0 else nc.gpsimd
                eng.dma_start(out_d[tb * P:(tb + 1) * P, :], o_sb[:])

            # ---- heads loop -------------------------------------------
            # 32 units per head (qh0 kc0-15, then qh1 kc0-15), scores
            # first in each unit, fillers quantized to <=430ns. Each
            # head's PV (both halves) runs one head later, evening the
            # per-head PE load (HAM downclocks an idling PE) and leaving
            # only head-3's half-1 PV + 8 output blocks after the last
            # exp. The ctx pool (bufs=1) serializes pv half -> normalize
            # -> next pv half naturally.

            def emit_head0():
                for u in range(32):
                    qh, kc = (0, u) if u < 16 else (1, u - 16)
                    scores_unit(0, kc, qh)
                    if u < 8:
                        proj_step(1, 0, 1, u)        # K0 second-half
                    elif u < 16:
                        proj_step(0, 0, 1, u - 8)    # Q0 second-half
                    v_step(u // 2, u % 2)            # V tb 0-15

            def emit_head1(ets0):
                for u in range(32):
                    qh, kc = (0, u) if u < 16 else (1, u - 16)
                    scores_unit(1, kc, qh)
                    if u < 8:
                        proj_step(0, 1, 0, u)        # Q1 first-half
                    elif u < 16:
                        proj_step(1, 1, 0, u - 8)    # K1 first-half
                    elif u < 24:
                        proj_step(1, 1, 1, u - 16)   # K1 second-half
                    else:
                        proj_step(0, 1, 1, u - 24)   # Q1 second-half
                    if u < 8:
                        pv_step(0, 0, u, ets=ets0[u])
                    elif u == 8:
                        normalize(0, 0, 0, 2)
                    elif u == 9:
                        normalize(0, 0, 1, 2)
                    elif u < 18:
                        pv_step(0, 1, u - 10, ets=ets0[u - 10])
                    elif u == 18:
                        normalize(0, 1, 0, 2)
                    elif u == 19:
                        normalize(0, 1, 1, 2)

            def emit_head2(ets1):
                for u in range(32):
                    qh, kc = (0, u) if u < 16 else (1, u - 16)
                    scores_unit(2, kc, qh)
                    if u < 8:
                        pv_step(1, 0, u, ets=ets1[u])
                    elif u == 8:
                        normalize(1, 0, 0, 2)
                    elif u == 9:
                        normalize(1, 0, 1, 2)
                    elif u < 18:
                        pv_step(1, 1, u - 10, ets=ets1[u - 10])
                    elif u == 18:
                        normalize(1, 1, 0, 2)
                    elif u == 19:
                        normalize(1, 1, 1, 2)
                    elif 20 <= u < 28:
                        pv_step(2, 0, u - 20)
                    elif u == 28:
                        normalize(2, 0, 0, 2)
                    elif u == 29:
                        normalize(2, 0, 1, 2)

            def emit_head3(ets2):
                for u in range(32):
                    qh, kc = (0, u) if u < 16 else (1, u - 16)
                    scores_unit(3, kc, qh)
                    if u < 8:
                        pv_step(2, 1, u, ets=ets2[u])
                    elif u == 8:
                        normalize(2, 1, 0, 2)
                    elif u == 9:
                        normalize(2, 1, 1, 2)
                    elif u < 18:
                        pv_step(3, 0, u - 10)
                    elif u < 22:
                        normalize(3, 0, u - 18, 4)
                    elif u < 30:
                        out_tb(u - 22, 0)

            emit_head0()
            ets0 = list(etps)
            emit_head1(ets0)
            ets1 = list(etps)
            emit_head2(ets1)
            ets2 = list(etps)
            emit_head3(ets2)
            ets3 = list(etps)

            # ---- tail: head-3 half-1 PV + normalize + out blocks 8-15.
            # Movers split between DVE (0) and the now-idle ACT (1).
            for pr in range(NPAIR):
                pv_step(3, 1, pr, ets=ets3[pr])
            normalize(3, 1, 0, 4)
            normalize(3, 1, 1, 4)
            out_tb(8, 0)
            out_tb(9, 1)
            normalize(3, 1, 2, 4)
            out_tb(10, 0)
            out_tb(11, 1)
            normalize(3, 1, 3, 4)
            out_tb(12, 0)
            out_tb(13, 1)
            out_tb(14, 0)
            out_tb(15, 1)

    nc.compile()
    return nc


def _get_compiled():
    if "nc" not in _CACHE:
        _CACHE["nc"] = _build()
    return _CACHE["nc"]


def kernel(x, Wq, bq, Wk, bk, Wv, bv, Wo, bo):
    from concourse.bass_utils import run_bass_kernel_spmd

    nc = _get_compiled()
    x = np.asarray(x, dtype=np.float32)
    Wq, bq = np.asarray(Wq, np.float32), np.asarray(bq, np.float32)
    Wk, bk = np.asarray(Wk, np.float32), np.asarray(bk, np.float32)
    Wv, bv = np.asarray(Wv, np.float32), np.asarray(bv, np.float32)
    Wo, bo = np.asarray(Wo, np.float32), np.asarray(bo, np.float32)

    bf = ml_dtypes.bfloat16
    in_maps = []
    for c in range(NCORES):
        b, g = c // 4, c % 4
        cols = slice(g * DL, (g + 1) * DL)
        bq_l, bk_l = bq[cols], bk[cols]
        bqk = np.stack(
            [bq_l[0:P], bq_l[P:2 * P], bk_l[0:P], bk_l[P:2 * P]], axis=1)
        in_maps.append({
            "xt": np.ascontiguousarray(x[b].T).astype(bf),
            "wq": Wq[:, cols].astype(bf),
            "wk": Wk[:, cols].astype(bf),
            "wv": Wv[:, cols].astype(bf),
            "wo": Wo[cols, :].astype(bf),
            "bqk": np.ascontiguousarray(bqk, np.float32),
        })

    _CACHE["in_maps"] = in_maps
    res = run_bass_kernel_spmd(nc, in_maps, list(range(NCORES)))

    const_row = bv.astype(np.float64) @ Wo.astype(np.float64) + bo
    out = np.zeros((B, S, D), np.float64)
    for c in range(NCORES):
        out[c // 4] += res.results[c]["out"].astype(np.float64)
    out += const_row
    return out.astype(np.float32)


# revision 12
# speedup vs baseline: 1.1673x; 1.1673x over previous
"""Multi-head attention (B=2, S=2048, D=1024, H=16) on 8 trn2 NeuronCores.

Sharding: core c -> batch b = c // 4, head group g = c % 4 (heads 4g..4g+3).
Each core computes, for its batch shard and 4 heads:
  QT/KT = (x W + b)^T in [d_local, seq] layout, V in [seq, d_local] layout,
  transposed scores S^T[k, q] = K Q^T (so softmax needs no transposes),
  exp via ACT (scale fused), fp8e4m3 DoubleRow PV with an appended ones
  column yielding both the unnormalized context and the softmax row sums,
  normalization via a gpsimd partition-broadcast reciprocal multiply, and
  a partial output projection against a row shard of Wo.
Host sums the 4 partials per batch and adds the constant row bv @ Wo + bo.

Schedule: the ACT engine (exp of 16.8M scores, ~1.11us per [128,1024]
unit) paces the kernel, and engines synchronize on cumulative completion
counters, so every PE instruction emitted between two scores units delays
the second unit's exp by its full duration. Hence:
  - per-head units are (token-half qh, key-chunk kc), scores first in
    each unit, fillers quantized to <=430ns pieces (half V projections,
    one projection chunk, 1-2 PV DoubleRow matmuls) so per-unit PE time
    stays at or under the ACT cadence;
  - PSUM: 4 banks of rotating [128,1024] scores/work tiles, 2 banks for
    the PV context accumulator (one token-half at a time - the exps of a
    whole head stay resident in fp8, bufs=16), and 2 banks for a
    dedicated projection-accumulator pool so the second-half/dblk1
    projections never wait on a normalize of the previous head;
  - PV for token-half 0 runs inside the head (pairs all ready by unit
    16); PV for half 1 + its normalize slide into the next head's first
    units; head 3's half-0 output blocks run inside head 3, so only 8
    output blocks + half-1 normalize remain after the last exp;
  - PSUM->SBUF output movers run on DVE in-head and alternate DVE/ACT in
    the tail; output DMA alternates the sync/gpsimd queues.
"""

import sys

sys.path.insert(0, "/opt/trn_rl_repo")

import numpy as np
import ml_dtypes

B = 2
S = 2048
D = 1024
H = 16
HD = 64
NCORES = 8
HPC = 4          # heads per core
DL = HPC * HD    # 256 local head dims per core
P = 128
KCH = S // P     # 16 key chunks
DCH = D // P     # 8 contraction chunks
TBLK = S // P    # 16 token blocks
NPAIR = KCH // 2
SCALE = 1.0 / np.sqrt(HD)

USE_FP8_PV = True

_CACHE = {}


def _build():
    import concourse.bass as bass  # noqa: F401
    import concourse.mybir as mybir
    import concourse.tile as tile
    from concourse import bacc

    bf16 = mybir.dt.bfloat16
    f32 = mybir.dt.float32
    fp8 = mybir.dt.float8e4
    DR = mybir.MatmulPerfMode.DoubleRow
    Exp = mybir.ActivationFunctionType.Exp

    nc = bacc.Bacc("TRN2", target_bir_lowering=False, debug=False,
                   num_devices=NCORES)

    xT_d = nc.dram_tensor("xt", [D, S], bf16, kind="ExternalInput")
    wq_d = nc.dram_tensor("wq", [D, DL], bf16, kind="ExternalInput")
    wk_d = nc.dram_tensor("wk", [D, DL], bf16, kind="ExternalInput")
    wv_d = nc.dram_tensor("wv", [D, DL], bf16, kind="ExternalInput")
    wo_d = nc.dram_tensor("wo", [DL, D], bf16, kind="ExternalInput")
    bqk_d = nc.dram_tensor("bqk", [P, 4], f32, kind="ExternalInput")
    out_d = nc.dram_tensor("out", [S, D], bf16, kind="ExternalOutput")

    att_dt = fp8 if USE_FP8_PV else bf16

    with tile.TileContext(nc) as tc:
        with (
            tc.tile_pool(name="persist", bufs=1) as pp,
            tc.tile_pool(name="stream", bufs=3) as sp,
            tc.tile_pool(name="psum", bufs=2, space="PSUM") as ps,
        ):
            # ---- input DMAs: weights first (small), then x token-halves.
            bqk_s = pp.tile([P, 4], f32, tag="bqk", name="bqk_s")
            nc.sync.dma_start(bqk_s[:], bqk_d[:])
            wq_s = pp.tile([P, DCH, DL], bf16, tag="wq", name="wq_s")
            wk_s = pp.tile([P, DCH, DL], bf16, tag="wk", name="wk_s")
            wv_s = pp.tile([P, DCH, DL], bf16, tag="wv", name="wv_s")
            xts = [pp.tile([P, S], bf16, tag=f"xt{c}", name=f"xt{c}")
                   for c in range(DCH)]
            # weight chunks interleaved with x first-halves so projection
            # matmuls can consume each chunk right as it lands
            nc.gpsimd.dma_start(wq_s[:, 0, :], wq_d[0:P, :])
            nc.sync.dma_start(wk_s[:, 0, :], wk_d[0:P, :])
            for c in range(DCH):
                eng = nc.sync if c % 2 == 0 else nc.gpsimd
                eng.dma_start(xts[c][:, 0:1024], xT_d[c * P:(c + 1) * P, 0:1024])
                if c + 1 < DCH:
                    nc.gpsimd.dma_start(wq_s[:, c + 1, :],
                                        wq_d[(c + 1) * P:(c + 2) * P, :])
                    nc.sync.dma_start(wk_s[:, c + 1, :],
                                      wk_d[(c + 1) * P:(c + 2) * P, :])
            for c in range(DCH):
                nc.gpsimd.dma_start(wv_s[:, c, :], wv_d[c * P:(c + 1) * P, :])
            for c in range(DCH):
                eng = nc.sync if c % 2 == 1 else nc.gpsimd
                eng.dma_start(xts[c][:, 1024:2048],
                              xT_d[c * P:(c + 1) * P, 1024:2048])
            wo_s = pp.tile([P, 2, D], bf16, tag="wo", name="wo_s")
            for dc in range(2):
                nc.gpsimd.dma_start(wo_s[:, dc, :], wo_d[dc * P:(dc + 1) * P, :])

            qt = [pp.tile([P, S], bf16, tag=f"qt{d}", name=f"qt{d}")
                  for d in range(2)]
            kt = [pp.tile([P, S], bf16, tag=f"kt{d}", name=f"kt{d}")
                  for d in range(2)]

            # ---- phase B: Q/K dblk0 first-token-half projections,
            # consuming each x half-chunk once on arrival.
            qacc = ps.tile([P, 1024], f32, tag="work", name="qacc_h0")
            kacc = ps.tile([P, 1024], f32, tag="work", name="kacc_h0")
            for c in range(DCH):
                for ns in range(2):
                    nc.tensor.matmul(
                        qacc[:, ns * 512:(ns + 1) * 512],
                        wq_s[:, c, 0:P],
                        xts[c][:, ns * 512:(ns + 1) * 512],
                        start=(c == 0), stop=(c == DCH - 1))
                for ns in range(2):
                    nc.tensor.matmul(
                        kacc[:, ns * 512:(ns + 1) * 512],
                        wk_s[:, c, 0:P],
                        xts[c][:, ns * 512:(ns + 1) * 512],
                        start=(c == 0), stop=(c == DCH - 1))
            nc.vector.tensor_scalar_add(qt[0][:, 0:1024], qacc[:],
                                        bqk_s[:, 0:1])
            nc.vector.tensor_scalar_add(kt[0][:, 0:1024], kacc[:],
                                        bqk_s[:, 2:3])

            # ---- projection-accumulator pool: its own 2 PSUM banks, one
            # proj-half at a time, 1 chunk (2 MMs) per unit.
            pj_ref = [None]

            def proj_step(which, dblk, half, c):
                w_s = wq_s if which == 0 else wk_s
                if c == 0:
                    pj_ref[0] = ps.tile([P, 1024], f32, tag="pacc", bufs=1,
                                        name=f"pacc{which}{dblk}{half}")
                acc = pj_ref[0]
                for ns in range(2):
                    nc.tensor.matmul(
                        acc[:, ns * 512:(ns + 1) * 512],
                        w_s[:, c, dblk * P:(dblk + 1) * P],
                        xts[c][:, half * 1024 + ns * 512:
                               half * 1024 + (ns + 1) * 512],
                        start=(c == 0), stop=(c == DCH - 1))
                if c == DCH - 1:
                    t_sb = qt[dblk] if which == 0 else kt[dblk]
                    bcol = dblk if which == 0 else 2 + dblk
                    nc.vector.tensor_scalar_add(
                        t_sb[:, half * 1024:(half + 1) * 1024],
                        acc[:], bqk_s[:, bcol:bcol + 1])

            # V tiles: k-chunk PAIRS [128, 2, 4 heads * 68]; col 68h+64 is
            # the softmax-sum ones column. Emitted as two 4-chunk quanta.
            vts = [None] * NPAIR
            v_ref = [None]

            def v_step(tb, quantum):
                pr, j = tb // 2, tb % 2
                if quantum == 0:
                    if j == 0:
                        vt = pp.tile([P, 2, HPC * 68], att_dt, tag=f"v{pr}",
                                     name=f"v{pr}")
                        v4 = vt.rearrange("p j (h e) -> p j h e", e=68)
                        nc.gpsimd.memset(v4[:, :, :, 64:65], 1.0)
                        vts[pr] = vt
                    v_ref[0] = ps.tile([P, 1024], f32, tag="work",
                                       name=f"ps_v{tb}")
                acc = v_ref[0]
                for kc in range(4 * quantum, 4 * quantum + 4):
                    nc.tensor.matmul(
                        acc[:, 0:DL],
                        xts[kc][:, tb * P:(tb + 1) * P],
                        wv_s[:, kc, :],
                        start=(kc == 0), stop=(kc == DCH - 1))
                if quantum == 1:
                    v4 = vts[pr].rearrange("p j (h e) -> p j h e", e=68)
                    nc.vector.tensor_copy(
                        v4[:, j, :, 0:64],
                        acc[:, 0:DL].rearrange("p (h e) -> p h e", e=64))

            etps = [None] * NPAIR
            ctx_ps = {}      # qh -> current [P,1024] ctx psum tile
            ctx_sb = [pp.tile([P, S], bf16, tag=f"ctx{dc}", name=f"ctx{dc}")
                      for dc in range(2)]

            def scores_unit(h, kc, qh):
                dblk = h // 2
                roff = 64 * (h % 2)
                pr, j = kc // 2, kc % 2
                if j == 0 and qh == 0:
                    # whole head resident: 16 buffers so PV of half 1 can
                    # trail into the next head without stalling allocs.
                    etps[pr] = sp.tile([P, 2, S], att_dt, tag="expt",
                                       bufs=16, name=f"expt{h}_{pr}")
                sc = ps.tile([P, 1024], f32, tag="work",
                             name=f"sc{h}_{kc}_{qh}")
                for ns in range(2):
                    nc.tensor.matmul(
                        sc[:, ns * 512:(ns + 1) * 512],
                        kt[dblk][roff:roff + 64, kc * P:(kc + 1) * P],
                        qt[dblk][roff:roff + 64,
                                 qh * 1024 + ns * 512:qh * 1024 + (ns + 1) * 512],
                        start=True, stop=True)
                nc.scalar.activation(
                    etps[pr][:, j, qh * 1024:(qh + 1) * 1024], sc[:],
                    Exp, scale=float(SCALE))

            def pv_step(h, qh, pr, ets=None):
                """One pair's PV for one token-half: 2 fp8 DR matmuls."""
                if pr == 0:
                    ctx_ps[qh] = ps.tile([P, 1024], f32, tag="ctx", bufs=1,
                                         name=f"ps_ctx{h}_{qh}")
                cps = ctx_ps[qh]
                v4 = vts[pr].rearrange("p j (h e) -> p j h e", e=68)
                et = ets if ets is not None else etps[pr]
                for ns in range(2):
                    nc.tensor.matmul(
                        cps[0:65, ns * 512:(ns + 1) * 512],
                        v4[:, :, h, 0:65],
                        et[:, :, qh * 1024 + ns * 512:
                           qh * 1024 + (ns + 1) * 512],
                        start=(pr == 0), stop=(pr == NPAIR - 1),
                        perf_mode=DR)

            def normalize(h, qh, part, nparts=2):
                """Normalize 1/nparts of head h's half-qh context."""
                dblk = h // 2
                roff = 64 * (h % 2)
                w = 1024 // nparts
                cps = ctx_ps[qh]
                hs = slice(part * w, (part + 1) * w)
                gs = slice(qh * 1024 + part * w, qh * 1024 + (part + 1) * w)
                srow = sp.tile([1, w], f32, tag=f"srow{w}", bufs=2,
                               name=f"srow{h}_{qh}_{part}")
                nc.vector.tensor_copy(srow[:], cps[64:65, hs])
                rec = sp.tile([1, w], f32, tag=f"rec{w}", bufs=2,
                              name=f"rec{h}_{qh}_{part}")
                nc.vector.reciprocal_approx_fast(rec[:], srow[:])
                bc = sp.tile([64, w], f32, tag=f"bc{w}", bufs=2,
                             name=f"bc{h}_{qh}_{part}")
                nc.gpsimd.partition_broadcast(bc[:], rec[:])
                nc.vector.tensor_mul(
                    ctx_sb[dblk][roff:roff + 64, gs],
                    cps[0:64, hs], bc[:])

            def out_tb(tb, mover):
                acc = ps.tile([P, 1024], f32, tag="work", name=f"ps_o{tb}")
                for dc in range(2):
                    for ns in range(2):
                        nc.tensor.matmul(
                            acc[:, ns * 512:(ns + 1) * 512],
                            ctx_sb[dc][:, tb * P:(tb + 1) * P],
                            wo_s[:, dc, ns * 512:(ns + 1) * 512],
                            start=(dc == 0), stop=(dc == 1))
                o_sb = sp.tile([P, D], bf16, tag="osb", name=f"osb{tb}")
                if mover == 0:
                    nc.vector.tensor_copy(o_sb[:], acc[:])
                else:
                    nc.scalar.copy(o_sb[:], acc[:])
                eng = nc.sync if tb % 2 == 0 else nc.gpsimd
                eng.dma_start(out_d[tb * P:(tb + 1) * P, :], o_sb[:])

            # ---- heads loop -------------------------------------------
            # 32 units per head (qh0 kc0-15, then qh1 kc0-15), scores
            # first in each unit. Real PE work (incl. ldweights and PSUM
            # bank-switch overhead) slightly exceeds the ACT exp span, so
            # units are packed to AT LEAST the ~1.11us ACT cadence -
            # underloaded units waste wall time through the 2-buffer
            # lockstep and let the HAM power manager downclock the PE.
            # Light units get a harmless dummy projection matmul burst
            # into the (idle) pacc bank to keep the clock up.

            dummy_ctr = [0]

            def dummy_fill(nmm=2):
                i = dummy_ctr[0]
                dummy_ctr[0] += 1
                acc = ps.tile([P, 1024], f32, tag="pacc", bufs=1,
                              name=f"dummy{i}")
                for ns in range(nmm):
                    nc.tensor.matmul(
                        acc[:, (ns % 2) * 512:(ns % 2 + 1) * 512],
                        wq_s[:, ns % DCH, 0:P],
                        xts[ns % DCH][:, 0:512],
                        start=True, stop=True)

            def emit_head0():
                for u in range(32):
                    qh, kc = (0, u) if u < 16 else (1, u - 16)
                    scores_unit(0, kc, qh)
                    if u < 8:
                        proj_step(1, 0, 1, u)        # K0 second-half
                    elif u < 16:
                        proj_step(0, 0, 1, u - 8)    # Q0 second-half
                    v_step(u // 2, u % 2)            # V tb 0-15

            def emit_head1(ets0):
                for u in range(32):
                    qh, kc = (0, u) if u < 16 else (1, u - 16)
                    scores_unit(1, kc, qh)
                    if u < 8:
                        proj_step(0, 1, 0, u)        # Q1 first-half
                        pv_step(0, 0, u, ets=ets0[u])
                    elif u == 8:
                        normalize(0, 0, 0, 2)
                        dummy_fill()
                    elif u == 9:
                        normalize(0, 0, 1, 2)
                        dummy_fill()
                    elif u < 18:
                        proj_step(1, 1, 0, u - 10)   # K1 first-half
                        pv_step(0, 1, u - 10, ets=ets0[u - 10])
                    elif u == 18:
                        normalize(0, 1, 0, 2)
                        dummy_fill()
                    elif u == 19:
                        normalize(0, 1, 1, 2)
                        dummy_fill()
                    elif u < 28:
                        proj_step(1, 1, 1, u - 20)   # K1 second-half
                        dummy_fill()
                    else:
                        dummy_fill(4)

            def emit_head2(ets1):
                for u in range(32):
                    qh, kc = (0, u) if u < 16 else (1, u - 16)
                    scores_unit(2, kc, qh)
                    if u < 8:
                        pv_step(1, 0, u, ets=ets1[u])
                        dummy_fill()
                    elif u == 8:
                        normalize(1, 0, 0, 2)
                        proj_step(0, 1, 1, 0)        # Q1 second-half
                    elif u == 9:
                        normalize(1, 0, 1, 2)
                        proj_step(0, 1, 1, 1)
                    elif u < 16:
                        proj_step(0, 1, 1, u - 8)
                        pv_step(1, 1, u - 10, ets=ets1[u - 10])
                    elif u < 18:
                        pv_step(1, 1, u - 10, ets=ets1[u - 10])
                        dummy_fill()
                    elif u == 18:
                        normalize(1, 1, 0, 2)
                        dummy_fill()
                    elif u == 19:
                        normalize(1, 1, 1, 2)
                        dummy_fill()
                    elif u < 28:
                        pv_step(2, 0, u - 20)
                        dummy_fill()
                    elif u == 28:
                        normalize(2, 0, 0, 2)
                        dummy_fill()
                    elif u == 29:
                        normalize(2, 0, 1, 2)
                        dummy_fill()
                    else:
                        dummy_fill(4)

            def emit_head3(ets2):
                for u in range(32):
                    qh, kc = (0, u) if u < 16 else (1, u - 16)
                    scores_unit(3, kc, qh)
                    if u < 8:
                        pv_step(2, 1, u, ets=ets2[u])
                        dummy_fill()
                    elif u == 8:
                        normalize(2, 1, 0, 2)
                        dummy_fill()
                    elif u == 9:
                        normalize(2, 1, 1, 2)
                        dummy_fill()
                    elif u < 18:
                        pv_step(3, 0, u - 10)
                        dummy_fill()
                    elif u < 22:
                        normalize(3, 0, u - 18, 4)
                    elif u < 30:
                        out_tb(u - 22, 0)
                    else:
                        dummy_fill(2)

            emit_head0()
            ets0 = list(etps)
            emit_head1(ets0)
            ets1 = list(etps)
            emit_head2(ets1)
            ets2 = list(etps)
            emit_head3(ets2)
            ets3 = list(etps)

            # ---- tail: head-3 half-1 PV + normalize + out blocks 8-15.
            for pr in range(NPAIR):
                pv_step(3, 1, pr, ets=ets3[pr])
            normalize(3, 1, 0, 4)
            normalize(3, 1, 1, 4)
            out_tb(8, 0)
            out_tb(9, 1)
            normalize(3, 1, 2, 4)
            out_tb(10, 0)
            out_tb(11, 1)
            normalize(3, 1, 3, 4)
            out_tb(12, 0)
            out_tb(13, 1)
            out_tb(14, 0)
            out_tb(15, 1)

    nc.compile()
    return nc


def _get_compiled():
    if "nc" not in _CACHE:
        _CACHE["nc"] = _build()
    return _CACHE["nc"]


def kernel(x, Wq, bq, Wk, bk, Wv, bv, Wo, bo):
    from concourse.bass_utils import run_bass_kernel_spmd

    nc = _get_compiled()
    x = np.asarray(x, dtype=np.float32)
    Wq, bq = np.asarray(Wq, np.float32), np.asarray(bq, np.float32)
    Wk, bk = np.asarray(Wk, np.float32), np.asarray(bk, np.float32)
    Wv, bv = np.asarray(Wv, np.float32), np.asarray(bv, np.float32)
    Wo, bo = np.asarray(Wo, np.float32), np.asarray(bo, np.float32)

    bf = ml_dtypes.bfloat16
    in_maps = []
    for c in range(NCORES):
        b, g = c // 4, c % 4
        cols = slice(g * DL, (g + 1) * DL)
        bq_l, bk_l = bq[cols], bk[cols]
        bqk = np.stack(
            [bq_l[0:P], bq_l[P:2 * P], bk_l[0:P], bk_l[P:2 * P]], axis=1)
        in_maps.append({
            "xt": np.ascontiguousarray(x[b].T).astype(bf),
            "wq": Wq[:, cols].astype(bf),
            "wk": Wk[:, cols].astype(bf),
            "wv": Wv[:, cols].astype(bf),
            "wo": Wo[cols, :].astype(bf),
            "bqk": np.ascontiguousarray(bqk, np.float32),
        })

    _CACHE["in_maps"] = in_maps
    res = run_bass_kernel_spmd(nc, in_maps, list(range(NCORES)))

    const_row = bv.astype(np.float64) @ Wo.astype(np.float64) + bo
    out = np.zeros((B, S, D), np.float64)
    for c in range(NCORES):
        out[c // 4] += res.results[c]["out"].astype(np.float64)
    out += const_row
    return out.astype(np.float32)
